# revision 2
# baseline (speedup 1.0000x reference)
"""FP8 semi-sparse activation linear kernel for Trainium2 (8 NeuronCores).

Computes: rowwise-fp8-quant(2:4-sparsify(relu(x)^2)) @ rowwise-fp8-quant(W).T -> bf16

Sharding: x rows split 4 ways (m-groups), W rows (= out cols) split 2 ways
(n-halves); core c handles m-group c % 4, n-half c // 4.

Key implementation notes:
  - TRN fp8e4 max is +-240 (vs OCP e4m3fn +-448).  We quantize to +-224
    (scale' = 2*scale_ref); powers of two commute with RNE so the fp8
    rounding grid matches the reference exactly (sans the denormal tail,
    which is ~2^-18 relative -- irrelevant).
  - The 2:4 sparsify runs on r = relu(x) (monotonic under squaring); the
    square is fused into the fp8 quantization via ACT's Square activation
    (out = Square(r * sqrt(inv))), which hardware evaluates exactly.
  - Transposes (K onto partitions) use the DMA xbar on bf16-bitcast fp8
    pairs: one dma_start_transpose per [128, 4096-fp8] tile.  Resulting
    layout: partition p, k-block b holds k = 256*b + 2*p + {0,1} as
    adjacent bytes.
  - The matmul runs in fp8 DoubleRow (2x) mode: the moving operand uses a
    [p, 2, n] AP (pair step 1 byte -- legal on the MM side), the
    stationary operand uses DoubleRowSwInterleave which expects adjacent
    A/B byte pairs with columns reversed; we pre-reverse x rows per
    128-tile on the host so PSUM rows come out in natural order.
  - Per-row x scales are computed on reversed rows; they are un-reversed
    on chip with a stream_shuffle (reverse within 32) + 4 partition-block
    DMA copies.
"""
import sys
import os

sys.path.insert(0, "/opt/trn_rl_repo")

import numpy as np
import ml_dtypes

import concourse.bass as bass
import concourse.mybir as mybir
from concourse.tile import TileContext
from concourse.bass_utils import run_bass_kernel_spmd

# ---------------------------------------------------------------------------
# Workaround: this environment's walrus rejects instructions with more than
# a couple of sync-wait conditions ("Too many sync wait commands").  Split
# excess waits onto NoOp instructions inserted before the offender.
import orjson as _orjson

_orig_to_json_bytes = bass.Bass.to_json_bytes
_LIMIT_DEFAULT = 1
_ws_counter = [0]


def _split_waits(doc):
    for fn in doc.get("functions", []):
        for blk in fn.get("blocks", []):
            insts = blk.get("instructions")
            if not insts:
                continue
            out = []
            changed = False
            for ins in insts:
                si = ins.get("sync_info")
                if si:
                    waits = si.get("on_wait") or []
                    if len(waits) > _LIMIT_DEFAULT:
                        excess = waits[:-_LIMIT_DEFAULT]
                        keep = waits[-_LIMIT_DEFAULT:]
                        for i in range(0, len(excess), _LIMIT_DEFAULT):
                            _ws_counter[0] += 1
                            out.append({
                                "name": f"I-waitsplit-{_ws_counter[0]}",
                                "engine": ins["engine"],
                                "opcode": "NoOp",
                                "ins": [],
                                "outs": [],
                                "sync_info": {
                                    "on_wait": excess[i:i + _LIMIT_DEFAULT],
                                    "on_update": [],
                                },
                            })
                        si["on_wait"] = keep
                        changed = True
                out.append(ins)
            if changed:
                blk["instructions"] = out
    return doc


def _patched_to_json_bytes(self):
    return _orjson.dumps(_split_waits(_orjson.loads(_orig_to_json_bytes(self))))


bass.Bass.to_json_bytes = _patched_to_json_bytes
# ---------------------------------------------------------------------------

F32 = mybir.dt.float32
FP8 = mybir.dt.float8e4
BF16 = mybir.dt.bfloat16
ALU = mybir.AluOpType
ACTF = mybir.ActivationFunctionType

M, K, NW = 8192, 4096, 4096
N_CORES = 8
MG, NH = 4, 2                  # m-groups x n-halves
MS, NS = M // MG, NW // NH     # 2048 x 2048 per-core output shard
NT = MS // 128                 # 16 x-tiles
WT = NS // 128                 # 16 w-tiles
KB = K // 256                  # 16 k-blocks of 256
SQRT224 = float(np.float32(np.sqrt(np.float32(224.0))))
INV224 = float(np.float32(1.0) / np.float32(224.0))


def _build_program():
    nc = bass.Bass()
    xs = nc.dram_tensor("xs", [MS, K], F32, kind="ExternalInput")
    ws = nc.dram_tensor("ws", [NS, K], F32, kind="ExternalInput")
    out = nc.dram_tensor("out", [MS, NS], BF16, kind="ExternalOutput")

    rev32 = list(range(31, -1, -1))

    with TileContext(nc) as tc:
        with tc.tile_pool(name="persist", bufs=1) as cpool, \
             tc.tile_pool(name="work", bufs=1) as pool, \
             tc.tile_pool(name="psum", bufs=8, space="PSUM") as psp:

            WqT = cpool.tile([128, KB, NS], BF16)     # 8 MB
            WscaleB = cpool.tile([128, NS], F32)      # 1 MB
            wsrow = cpool.tile([1, NS], F32)
            ones1 = cpool.tile([1, 128], F32)
            nc.vector.memset(ones1[:], 1.0)

            # ---------------- W prep ----------------
            for wt in range(WT):
                wtile = pool.tile([128, K], F32, tag="bigf32")
                nc.sync.dma_start(out=wtile[:], in_=ws[wt * 128:(wt + 1) * 128])
                wabs = pool.tile([128, 1], F32, tag="sA")
                nc.vector.tensor_reduce(out=wabs[:], in_=wtile[:],
                                        axis=mybir.AxisListType.X, op=ALU.max,
                                        apply_absolute_value=True)
                winv = pool.tile([128, 1], F32, tag="sB")
                nc.vector.reciprocal(out=winv[:], in_=wabs[:])
                winv2 = pool.tile([128, 1], F32, tag="sC")
                nc.vector.tensor_scalar_mul(out=winv2[:], in0=winv[:], scalar1=224.0)
                wscale = pool.tile([128, 1], F32, tag="sD")
                nc.vector.tensor_scalar_mul(out=wscale[:], in0=wabs[:], scalar1=INV224)
                wq = pool.tile([128, K], FP8, tag="q8")
                nc.scalar.activation(out=wq[:], in_=wtile[:], func=ACTF.Copy,
                                     scale=winv2[:])
                nc.sync.dma_start_transpose(WqT[:, :, wt * 128:(wt + 1) * 128],
                                            wq[:].bitcast(BF16))
                nc.sync.dma_start(out=wsrow[0:1, wt * 128:(wt + 1) * 128],
                                  in_=wscale[:])

            # broadcast W scales across partitions via ones-matmul
            for ch in range(4):
                psb = psp.tile([128, 512], F32, tag="acc")
                nc.tensor.matmul(psb[:], ones1[:],
                                 wsrow[0:1, ch * 512:(ch + 1) * 512],
                                 start=True, stop=True)
                nc.vector.tensor_copy(out=WscaleB[:, ch * 512:(ch + 1) * 512],
                                      in_=psb[:])

            # ---------------- X loop ----------------
            for mt in range(NT):
                xt = pool.tile([128, K], F32, tag="bigf32")
                nc.sync.dma_start(out=xt[:], in_=xs[mt * 128:(mt + 1) * 128])
                r = pool.tile([128, K], F32, tag="r")
                nc.scalar.activation(out=r[:], in_=xt[:], func=ACTF.Relu)

                # 2:4 threshold (2nd largest of each group of 4)
                r2 = r[:].rearrange("p (g two) -> p g two", two=2)
                pr = pool.tile([128, K // 2], F32, tag="pr")
                qs = pool.tile([128, K // 2], F32, tag="qs")
                nc.vector.tensor_tensor(out=pr[:], in0=r2[:, :, 0], in1=r2[:, :, 1], op=ALU.max)
                nc.vector.tensor_tensor(out=qs[:], in0=r2[:, :, 0], in1=r2[:, :, 1], op=ALU.min)
                pr2 = pr[:].rearrange("p (g two) -> p g two", two=2)
                qs2 = qs[:].rearrange("p (g two) -> p g two", two=2)
                u1 = pool.tile([128, K // 4], F32, tag="u1")
                u2 = pool.tile([128, K // 4], F32, tag="u2")
                thr = pool.tile([128, K // 4], F32, tag="thr")
                nc.vector.tensor_tensor(out=u1[:], in0=pr2[:, :, 0], in1=pr2[:, :, 1], op=ALU.min)
                nc.vector.tensor_tensor(out=u2[:], in0=qs2[:, :, 0], in1=qs2[:, :, 1], op=ALU.max)
                nc.vector.tensor_tensor(out=thr[:], in0=u1[:], in1=u2[:], op=ALU.max)

                # row max of kept values == row max of r == max(pr)
                rmax = pool.tile([128, 1], F32, tag="sA")
                nc.vector.tensor_reduce(out=rmax[:], in_=pr[:],
                                        axis=mybir.AxisListType.X, op=ALU.max)

                # mask + apply
                mask = pool.tile([128, K], F32, tag="mask")
                r4 = r[:].rearrange("p (g four) -> p g four", four=4)
                m4 = mask[:].rearrange("p (g four) -> p g four", four=4)
                for i in range(4):
                    nc.vector.tensor_tensor(out=m4[:, :, i], in0=r4[:, :, i],
                                            in1=thr[:], op=ALU.is_ge)
                nc.vector.tensor_tensor(out=r[:], in0=r[:], in1=mask[:], op=ALU.mult)

                # quant scale s = sqrt(224) / rmax ; dequant xsc = rmax^2 / 224
                rm2 = pool.tile([128, 1], F32, tag="sB")
                nc.vector.tensor_scalar_max(out=rm2[:], in0=rmax[:], scalar1=1e-5)
                rrec = pool.tile([128, 1], F32, tag="sC")
                nc.vector.reciprocal(out=rrec[:], in_=rm2[:])
                sq = pool.tile([128, 1], F32, tag="sD")
                nc.vector.tensor_scalar_mul(out=sq[:], in0=rrec[:], scalar1=SQRT224)
                xsc = pool.tile([128, 1], F32, tag="sE")
                nc.vector.tensor_tensor(out=xsc[:], in0=rmax[:], in1=rmax[:], op=ALU.mult)
                xsc2 = pool.tile([128, 1], F32, tag="sF")
                nc.vector.tensor_scalar_mul(out=xsc2[:], in0=xsc[:], scalar1=INV224)

                xq = pool.tile([128, K], FP8, tag="q8")
                nc.scalar.activation(out=xq[:], in_=r[:], func=ACTF.Square, scale=sq[:])
                xqT = pool.tile([128, KB, 128], BF16, tag="xqT")
                nc.sync.dma_start_transpose(xqT[:], xq[:].bitcast(BF16))

                # un-reverse per-row scales (x rows are host-reversed per tile)
                xsh = pool.tile([128, 1], F32, tag="sG")
                nc.vector.stream_shuffle(out=xsh[:], in_=xsc2[:], mask=rev32)
                xnat = pool.tile([128, 1], F32, tag="sH")
                for q in range(4):
                    nc.sync.dma_start(out=xnat[32 * (3 - q):32 * (4 - q)],
                                      in_=xsh[32 * q:32 * (q + 1)])

                # matmuls: 16 k-blocks x 4 n-chunks, DoubleRow fp8
                accs = [psp.tile([128, 512], F32, tag="acc", name=f"acc_{mt}_{ch}")
                        for ch in range(4)]
                wq8 = WqT[:].bitcast(FP8)  # [128, KB, 2*NS]
                xq8 = xqT[:].bitcast(FP8)  # [128, KB, 256]
                for blk in range(KB):
                    lhs = xq8[:, blk, :]
                    for ch in range(4):
                        rhs = wq8[:, blk, ch * 1024:(ch + 1) * 1024].rearrange(
                            "p (n two) -> p two n", two=2)
                        nc.tensor.matmul(accs[ch][:], lhs, rhs,
                                         start=(blk == 0), stop=(blk == KB - 1),
                                         perf_mode=mybir.MatmulPerfMode.DoubleRowSwInterleave)

                # dequant + store
                ost = pool.tile([128, NS], BF16, tag="ost")
                for ch in range(4):
                    tmp = pool.tile([128, 512], F32, tag="tmp")
                    nc.scalar.activation(out=tmp[:], in_=accs[ch][:], func=ACTF.Copy,
                                         scale=xnat[:])
                    nc.vector.tensor_tensor(out=ost[:, ch * 512:(ch + 1) * 512],
                                            in0=tmp[:],
                                            in1=WscaleB[:, ch * 512:(ch + 1) * 512],
                                            op=ALU.mult)
                nc.sync.dma_start(out=out[mt * 128:(mt + 1) * 128], in_=ost[:])

    return nc


_cached_nc = None


def _get_nc():
    global _cached_nc
    if _cached_nc is None:
        _cached_nc = _build_program()
    return _cached_nc


def _run(x, W, trace=False):
    x = np.ascontiguousarray(x, dtype=np.float32)
    W = np.ascontiguousarray(W, dtype=np.float32)
    assert x.shape == (M, K) and W.shape == (NW, K)
    nc = _get_nc()
    in_maps = []
    for c in range(N_CORES):
        g, h = c % MG, c // MG
        xsh = x[g * MS:(g + 1) * MS].reshape(NT, 128, K)[:, ::-1, :].reshape(MS, K)
        in_maps.append({
            "xs": np.ascontiguousarray(xsh),
            "ws": W[h * NS:(h + 1) * NS],
        })
    res = run_bass_kernel_spmd(nc, in_maps, core_ids=list(range(N_CORES)),
                               trace=trace)
    outf = np.empty((M, NW), dtype=ml_dtypes.bfloat16)
    for c in range(N_CORES):
        g, h = c % MG, c // MG
        outf[g * MS:(g + 1) * MS, h * NS:(h + 1) * NS] = res.results[c]["out"]
    return outf, res


def kernel(x, W):
    out, _ = _run(x, W, trace=False)
    return out


# revision 3
# speedup vs baseline: 1.2251x; 1.2251x over previous
"""FP8 semi-sparse activation linear kernel for Trainium2 (8 NeuronCores).

Computes: rowwise-fp8-quant(2:4-sparsify(relu(x)^2)) @ rowwise-fp8-quant(W).T -> bf16

Sharding: x rows split 4 ways (m-groups), W rows (= out cols) split 2 ways
(n-halves); core c handles m-group c % 4, n-half c // 4.

Key implementation notes:
  - TRN fp8e4 max is +-240 (vs OCP e4m3fn +-448).  We quantize to +-224
    (scale' = 2*scale_ref); powers of two commute with RNE so the fp8
    rounding grid matches the reference exactly (sans the denormal tail,
    which is ~2^-18 relative -- irrelevant).
  - The 2:4 sparsify runs on r = relu(x) (monotonic under squaring); the
    square is fused into the fp8 quantization via ACT's Square activation
    (out = Square(r * sqrt(inv))), which hardware evaluates exactly.
  - Transposes (K onto partitions) use the DMA xbar on bf16-bitcast fp8
    pairs: one dma_start_transpose per [128, 4096-fp8] tile.  Resulting
    layout: partition p, k-block b holds k = 256*b + 2*p + {0,1} as
    adjacent bytes.
  - The matmul runs in fp8 DoubleRow (2x) mode: the moving operand uses a
    [p, 2, n] AP (pair step 1 byte -- legal on the MM side), the
    stationary operand uses DoubleRowSwInterleave which expects adjacent
    A/B byte pairs with columns reversed; we pre-reverse x rows per
    128-tile on the host so PSUM rows come out in natural order.
  - Per-row x scales are computed on reversed rows; they are un-reversed
    on chip with a stream_shuffle (reverse within 32) + 4 partition-block
    DMA copies.
"""
import sys
import os

sys.path.insert(0, "/opt/trn_rl_repo")

import numpy as np
import ml_dtypes

import concourse.bass as bass
import concourse.mybir as mybir
from concourse.tile import TileContext
from concourse.bass_utils import run_bass_kernel_spmd

# ---------------------------------------------------------------------------
# Workaround: this environment's walrus rejects instructions with more than
# a couple of sync-wait conditions ("Too many sync wait commands").  Split
# excess waits onto NoOp instructions inserted before the offender.
import orjson as _orjson

_orig_to_json_bytes = bass.Bass.to_json_bytes
_LIMIT_DEFAULT = 1
_ws_counter = [0]


def _split_waits(doc):
    for fn in doc.get("functions", []):
        for blk in fn.get("blocks", []):
            insts = blk.get("instructions")
            if not insts:
                continue
            out = []
            changed = False
            for ins in insts:
                si = ins.get("sync_info")
                if si:
                    waits = si.get("on_wait") or []
                    if len(waits) > _LIMIT_DEFAULT:
                        excess = waits[:-_LIMIT_DEFAULT]
                        keep = waits[-_LIMIT_DEFAULT:]
                        for i in range(0, len(excess), _LIMIT_DEFAULT):
                            _ws_counter[0] += 1
                            out.append({
                                "name": f"I-waitsplit-{_ws_counter[0]}",
                                "engine": ins["engine"],
                                "opcode": "NoOp",
                                "ins": [],
                                "outs": [],
                                "sync_info": {
                                    "on_wait": excess[i:i + _LIMIT_DEFAULT],
                                    "on_update": [],
                                },
                            })
                        si["on_wait"] = keep
                        changed = True
                out.append(ins)
            if changed:
                blk["instructions"] = out
    return doc


def _patched_to_json_bytes(self):
    return _orjson.dumps(_split_waits(_orjson.loads(_orig_to_json_bytes(self))))


bass.Bass.to_json_bytes = _patched_to_json_bytes
# ---------------------------------------------------------------------------

F32 = mybir.dt.float32
FP8 = mybir.dt.float8e4
BF16 = mybir.dt.bfloat16
ALU = mybir.AluOpType
ACTF = mybir.ActivationFunctionType

M, K, NW = 8192, 4096, 4096
N_CORES = 8
MG, NH = 4, 2                  # m-groups x n-halves
MS, NS = M // MG, NW // NH     # 2048 x 2048 per-core output shard
NT = MS // 128                 # 16 x-tiles
WT = NS // 128                 # 16 w-tiles
KB = K // 256                  # 16 k-blocks of 256
SQRT224 = float(np.float32(np.sqrt(np.float32(224.0))))
INV224 = float(np.float32(1.0) / np.float32(224.0))


def _build_program():
    nc = bass.Bass()
    xs = nc.dram_tensor("xs", [MS, K], F32, kind="ExternalInput")
    ws = nc.dram_tensor("ws", [NS, K], F32, kind="ExternalInput")
    out = nc.dram_tensor("out", [MS, NS], BF16, kind="ExternalOutput")

    rev32 = list(range(31, -1, -1))

    with TileContext(nc) as tc:
        with tc.tile_pool(name="persist", bufs=1) as cpool, \
             tc.tile_pool(name="work", bufs=1) as pool, \
             tc.tile_pool(name="psum", bufs=8, space="PSUM") as psp:

            WqT = cpool.tile([128, KB, NS], BF16)     # 8 MB
            WscaleB = cpool.tile([128, NS], F32)      # 1 MB
            wsrow = cpool.tile([1, NS], F32)
            ones1 = cpool.tile([1, 128], F32)
            nc.vector.memset(ones1[:], 1.0)

            # ---------------- W prep ----------------
            for wt in range(WT):
                wtile = pool.tile([128, K], F32, tag="bigf32")
                nc.sync.dma_start(out=wtile[:], in_=ws[wt * 128:(wt + 1) * 128])
                wabs = pool.tile([128, 1], F32, tag="sA")
                nc.vector.tensor_reduce(out=wabs[:], in_=wtile[:],
                                        axis=mybir.AxisListType.X, op=ALU.max,
                                        apply_absolute_value=True)
                winv = pool.tile([128, 1], F32, tag="sB")
                nc.vector.reciprocal(out=winv[:], in_=wabs[:])
                winv2 = pool.tile([128, 1], F32, tag="sC")
                nc.vector.tensor_scalar_mul(out=winv2[:], in0=winv[:], scalar1=224.0)
                wscale = pool.tile([128, 1], F32, tag="sD")
                nc.vector.tensor_scalar_mul(out=wscale[:], in0=wabs[:], scalar1=INV224)
                wq = pool.tile([128, K], FP8, tag="q8")
                nc.scalar.activation(out=wq[:], in_=wtile[:], func=ACTF.Copy,
                                     scale=winv2[:])
                nc.sync.dma_start_transpose(WqT[:, :, wt * 128:(wt + 1) * 128],
                                            wq[:].bitcast(BF16))
                nc.sync.dma_start(out=wsrow[0:1, wt * 128:(wt + 1) * 128],
                                  in_=wscale[:])

            # broadcast W scales across partitions via ones-matmul
            for ch in range(4):
                psb = psp.tile([128, 512], F32, tag="acc")
                nc.tensor.matmul(psb[:], ones1[:],
                                 wsrow[0:1, ch * 512:(ch + 1) * 512],
                                 start=True, stop=True)
                nc.vector.tensor_copy(out=WscaleB[:, ch * 512:(ch + 1) * 512],
                                      in_=psb[:])

            # ---------------- X loop ----------------
            for mt in range(NT):
                xt = pool.tile([128, K], F32, tag="bigf32")
                nc.sync.dma_start(out=xt[:], in_=xs[mt * 128:(mt + 1) * 128])
                r = pool.tile([128, K], F32, tag="r", bufs=2)
                nc.scalar.activation(out=r[:], in_=xt[:], func=ACTF.Relu)

                # 2:4 threshold (2nd largest of each group of 4)
                r2 = r[:].rearrange("p (g two) -> p g two", two=2)
                pr = pool.tile([128, K // 2], F32, tag="pr")
                qs = pool.tile([128, K // 2], F32, tag="qs")
                nc.vector.tensor_tensor(out=pr[:], in0=r2[:, :, 0], in1=r2[:, :, 1], op=ALU.max)
                nc.vector.tensor_tensor(out=qs[:], in0=r2[:, :, 0], in1=r2[:, :, 1], op=ALU.min)
                pr2 = pr[:].rearrange("p (g two) -> p g two", two=2)
                qs2 = qs[:].rearrange("p (g two) -> p g two", two=2)
                u1 = pool.tile([128, K // 4], F32, tag="u1")
                u2 = pool.tile([128, K // 4], F32, tag="u2")
                thr = pool.tile([128, K // 4], F32, tag="thr")
                nc.vector.tensor_tensor(out=u1[:], in0=pr2[:, :, 0], in1=pr2[:, :, 1], op=ALU.min)
                nc.vector.tensor_tensor(out=u2[:], in0=qs2[:, :, 0], in1=qs2[:, :, 1], op=ALU.max)
                nc.vector.tensor_tensor(out=thr[:], in0=u1[:], in1=u2[:], op=ALU.max)

                # row max of kept values == row max of r == max(pr)
                rmax = pool.tile([128, 1], F32, tag="sA")
                nc.vector.tensor_reduce(out=rmax[:], in_=pr[:],
                                        axis=mybir.AxisListType.X, op=ALU.max)

                # mask + apply
                mask = pool.tile([128, K], BF16, tag="mask")
                r4 = r[:].rearrange("p (g four) -> p g four", four=4)
                m4 = mask[:].rearrange("p (g four) -> p g four", four=4)
                for i in range(4):
                    nc.vector.tensor_tensor(out=m4[:, :, i], in0=r4[:, :, i],
                                            in1=thr[:], op=ALU.is_ge)
                nc.vector.tensor_tensor(out=r[:], in0=r[:], in1=mask[:], op=ALU.mult)

                # quant scale s = sqrt(224) / rmax ; dequant xsc = rmax^2 / 224
                rm2 = pool.tile([128, 1], F32, tag="sB")
                nc.vector.tensor_scalar_max(out=rm2[:], in0=rmax[:], scalar1=1e-5)
                rrec = pool.tile([128, 1], F32, tag="sC")
                nc.vector.reciprocal(out=rrec[:], in_=rm2[:])
                sq = pool.tile([128, 1], F32, tag="sD")
                nc.vector.tensor_scalar_mul(out=sq[:], in0=rrec[:], scalar1=SQRT224)
                xsc = pool.tile([128, 1], F32, tag="sE")
                nc.vector.tensor_tensor(out=xsc[:], in0=rmax[:], in1=rmax[:], op=ALU.mult)
                xsc2 = pool.tile([128, 1], F32, tag="sF")
                nc.vector.tensor_scalar_mul(out=xsc2[:], in0=xsc[:], scalar1=INV224)

                xq = pool.tile([128, K], FP8, tag="q8")
                nc.scalar.activation(out=xq[:], in_=r[:], func=ACTF.Square, scale=sq[:])
                xqT = pool.tile([128, KB, 128], BF16, tag="xqT")
                nc.sync.dma_start_transpose(xqT[:], xq[:].bitcast(BF16))

                # un-reverse per-row scales (x rows are host-reversed per tile)
                xsh = pool.tile([128, 1], F32, tag="sG")
                nc.vector.stream_shuffle(out=xsh[:], in_=xsc2[:], mask=rev32)
                xnat = pool.tile([128, 1], F32, tag="sH")
                for q in range(4):
                    nc.sync.dma_start(out=xnat[32 * (3 - q):32 * (4 - q)],
                                      in_=xsh[32 * q:32 * (q + 1)])

                # matmuls: 16 k-blocks x 4 n-chunks, DoubleRow fp8
                accs = [psp.tile([128, 512], F32, tag="acc", name=f"acc_{mt}_{ch}")
                        for ch in range(4)]
                wq8 = WqT[:].bitcast(FP8)  # [128, KB, 2*NS]
                xq8 = xqT[:].bitcast(FP8)  # [128, KB, 256]
                for blk in range(KB):
                    lhs = xq8[:, blk, :]
                    for ch in range(4):
                        rhs = wq8[:, blk, ch * 1024:(ch + 1) * 1024].rearrange(
                            "p (n two) -> p two n", two=2)
                        nc.tensor.matmul(accs[ch][:], lhs, rhs,
                                         start=(blk == 0), stop=(blk == KB - 1),
                                         perf_mode=mybir.MatmulPerfMode.DoubleRowSwInterleave)

                # dequant + store
                ost = pool.tile([128, NS], BF16, tag="ost")
                for ch in range(4):
                    tmp = pool.tile([128, 512], F32, tag="tmp")
                    nc.scalar.activation(out=tmp[:], in_=accs[ch][:], func=ACTF.Copy,
                                         scale=xnat[:])
                    nc.vector.tensor_tensor(out=ost[:, ch * 512:(ch + 1) * 512],
                                            in0=tmp[:],
                                            in1=WscaleB[:, ch * 512:(ch + 1) * 512],
                                            op=ALU.mult)
                nc.sync.dma_start(out=out[mt * 128:(mt + 1) * 128], in_=ost[:])

    return nc


_cached_nc = None


def _get_nc():
    global _cached_nc
    if _cached_nc is None:
        _cached_nc = _build_program()
    return _cached_nc


def _run(x, W, trace=False):
    x = np.ascontiguousarray(x, dtype=np.float32)
    W = np.ascontiguousarray(W, dtype=np.float32)
    assert x.shape == (M, K) and W.shape == (NW, K)
    nc = _get_nc()
    in_maps = []
    for c in range(N_CORES):
        g, h = c % MG, c // MG
        xsh = x[g * MS:(g + 1) * MS].reshape(NT, 128, K)[:, ::-1, :].reshape(MS, K)
        in_maps.append({
            "xs": np.ascontiguousarray(xsh),
            "ws": W[h * NS:(h + 1) * NS],
        })
    res = run_bass_kernel_spmd(nc, in_maps, core_ids=list(range(N_CORES)),
                               trace=trace)
    outf = np.empty((M, NW), dtype=ml_dtypes.bfloat16)
    for c in range(N_CORES):
        g, h = c % MG, c // MG
        outf[g * MS:(g + 1) * MS, h * NS:(h + 1) * NS] = res.results[c]["out"]
    return outf, res


def kernel(x, W):
    out, _ = _run(x, W, trace=False)
    return out


# revision 7
# speedup vs baseline: 1.6270x; 1.3280x over previous
"""FP8 semi-sparse activation linear kernel for Trainium2 (8 NeuronCores).

Computes: rowwise-fp8-quant(2:4-sparsify(relu(x)^2)) @ rowwise-fp8-quant(W).T -> bf16

Sharding: x rows split 4 ways (m-groups), W rows (= out cols) split 2 ways
(n-halves); core c handles m-group c % 4, n-half c // 4.

Key implementation notes:
  - TRN fp8e4 max is +-240 (vs OCP e4m3fn +-448).  We quantize to +-224
    (scale' = 2*scale_ref); powers of two commute with RNE so the fp8
    rounding grid matches the reference exactly (sans the denormal tail,
    which is ~2^-18 relative -- irrelevant).
  - The 2:4 sparsify runs on r = relu(x) (monotonic under squaring); the
    square is fused into the fp8 quantization via ACT's Square activation
    (out = Square(r * sqrt(inv))), which hardware evaluates exactly.
  - Transposes (K onto partitions) use the DMA xbar on bf16-bitcast fp8
    pairs: one dma_start_transpose per [128, 4096-fp8] tile.  Resulting
    layout: partition p, k-block b holds k = 256*b + 2*p + {0,1} as
    adjacent bytes.
  - The matmul runs in fp8 DoubleRow (2x) mode: the moving operand uses a
    [p, 2, n] AP (pair step 1 byte -- legal on the MM side), the
    stationary operand uses DoubleRowSwInterleave which expects adjacent
    A/B byte pairs with columns reversed; we pre-reverse x rows per
    128-tile on the host so PSUM rows come out in natural order.
  - Per-row x scales are computed on reversed rows; they are un-reversed
    on chip with a stream_shuffle (reverse within 32) + 4 partition-block
    DMA copies.
"""
import sys
import os

sys.path.insert(0, "/opt/trn_rl_repo")

import numpy as np
import ml_dtypes

import concourse.bass as bass
import concourse.mybir as mybir
from concourse.tile import TileContext
from concourse.bass_utils import run_bass_kernel_spmd

# ---------------------------------------------------------------------------
# Workaround: this environment's walrus rejects instructions with more than
# a couple of sync-wait conditions ("Too many sync wait commands").  Split
# excess waits onto NoOp instructions inserted before the offender.
import orjson as _orjson

_orig_to_json_bytes = bass.Bass.to_json_bytes
_LIMIT_DEFAULT = 1
_ws_counter = [0]


def _split_waits(doc):
    for fn in doc.get("functions", []):
        for blk in fn.get("blocks", []):
            insts = blk.get("instructions")
            if not insts:
                continue
            out = []
            changed = False
            for ins in insts:
                si = ins.get("sync_info")
                if si:
                    waits = si.get("on_wait") or []
                    if len(waits) > _LIMIT_DEFAULT:
                        excess = waits[:-_LIMIT_DEFAULT]
                        keep = waits[-_LIMIT_DEFAULT:]
                        for i in range(0, len(excess), _LIMIT_DEFAULT):
                            _ws_counter[0] += 1
                            out.append({
                                "name": f"I-waitsplit-{_ws_counter[0]}",
                                "engine": ins["engine"],
                                "opcode": "NoOp",
                                "ins": [],
                                "outs": [],
                                "sync_info": {
                                    "on_wait": excess[i:i + _LIMIT_DEFAULT],
                                    "on_update": [],
                                },
                            })
                        si["on_wait"] = keep
                        changed = True
                out.append(ins)
            if changed:
                blk["instructions"] = out
    return doc


def _patched_to_json_bytes(self):
    return _orjson.dumps(_split_waits(_orjson.loads(_orig_to_json_bytes(self))))


bass.Bass.to_json_bytes = _patched_to_json_bytes
# ---------------------------------------------------------------------------

F32 = mybir.dt.float32
FP8 = mybir.dt.float8e4
BF16 = mybir.dt.bfloat16
ALU = mybir.AluOpType
ACTF = mybir.ActivationFunctionType

M, K, NW = 8192, 4096, 4096
N_CORES = 8
MG, NH = 4, 2                  # m-groups x n-halves
MS, NS = M // MG, NW // NH     # 2048 x 2048 per-core output shard
NT = MS // 128                 # 16 x-tiles
WT = NS // 128                 # 16 w-tiles
KB = K // 256                  # 16 k-blocks of 256
SQRT224 = float(np.float32(np.sqrt(np.float32(224.0))))
INV224 = float(np.float32(1.0) / np.float32(224.0))


def _build_program():
    nc = bass.Bass()
    xs = nc.dram_tensor("xs", [MS, K], F32, kind="ExternalInput")
    ws = nc.dram_tensor("ws", [NS, K], F32, kind="ExternalInput")
    out = nc.dram_tensor("out", [MS, NS], BF16, kind="ExternalOutput")

    rev32 = list(range(31, -1, -1))

    with TileContext(nc) as tc:
        with tc.tile_pool(name="persist", bufs=1) as cpool, \
             tc.tile_pool(name="work", bufs=1) as pool, \
             tc.tile_pool(name="psum", bufs=8, space="PSUM") as psp:

            WqT = cpool.tile([128, KB, NS], BF16)     # 8 MB
            WscaleB = cpool.tile([128, NS], F32)      # 1 MB
            wsrow = cpool.tile([1, NS], F32)
            ones1 = cpool.tile([1, 128], F32)
            nc.vector.memset(ones1[:], 1.0)

            # NOTE: every DMA (loads, xbar transposes, stores, small copies)
            # is issued from nc.sync -- concurrent DMA on another ring
            # corrupts in-flight xbar transposes (HW bug, verified).

            def w_chain(wt):
                wtile = pool.tile([128, K], F32, tag="wtile", bufs=2,
                                  name=f"wtile_{wt}")
                nc.sync.dma_start(out=wtile[:], in_=ws[wt * 128:(wt + 1) * 128])
                wabs = pool.tile([128, 1], F32, tag="sA", bufs=2, name=f"wabs_{wt}")
                nc.vector.tensor_reduce(out=wabs[:], in_=wtile[:],
                                        axis=mybir.AxisListType.X, op=ALU.max,
                                        apply_absolute_value=True)
                winv = pool.tile([128, 1], F32, tag="sB", bufs=2, name=f"winv_{wt}")
                nc.vector.reciprocal(out=winv[:], in_=wabs[:])
                winv2 = pool.tile([128, 1], F32, tag="sC", bufs=2, name=f"winv2_{wt}")
                nc.vector.tensor_scalar_mul(out=winv2[:], in0=winv[:], scalar1=224.0)
                wscale = pool.tile([128, 1], F32, tag="sD", bufs=2, name=f"wscale_{wt}")
                nc.vector.tensor_scalar_mul(out=wscale[:], in0=wabs[:], scalar1=INV224)
                wq = pool.tile([128, K], FP8, tag="wq8", bufs=1, name=f"wq_{wt}")
                nc.scalar.activation(out=wq[:], in_=wtile[:], func=ACTF.Copy,
                                     scale=winv2[:])
                nc.sync.dma_start_transpose(WqT[:, :, wt * 128:(wt + 1) * 128],
                                            wq[:].bitcast(BF16))
                nc.sync.dma_start(out=wsrow[0:1, wt * 128:(wt + 1) * 128],
                                  in_=wscale[:])

            xts = {}
            xqs = {}
            xqts = {}
            xshs = {}
            xnats = {}

            def x_pre(mt):
                xt = pool.tile([128, K], F32, tag="xt", bufs=2, name=f"xt_{mt}")
                nc.sync.dma_start(out=xt[:], in_=xs[mt * 128:(mt + 1) * 128])
                nc.scalar.activation(out=xt[:], in_=xt[:], func=ACTF.Relu)
                xts[mt] = xt

            def x_post(mt):
                r = xts[mt]
                r2 = r[:].rearrange("p (g two) -> p g two", two=2)
                pr = pool.tile([128, K // 2], F32, tag="pr", name=f"pr_{mt}")
                qs = pool.tile([128, K // 2], F32, tag="qs", name=f"qs_{mt}")
                nc.vector.tensor_tensor(out=pr[:], in0=r2[:, :, 0], in1=r2[:, :, 1], op=ALU.max)
                nc.vector.tensor_tensor(out=qs[:], in0=r2[:, :, 0], in1=r2[:, :, 1], op=ALU.min)
                pr2 = pr[:].rearrange("p (g two) -> p g two", two=2)
                qs2 = qs[:].rearrange("p (g two) -> p g two", two=2)
                u1 = pool.tile([128, K // 4], F32, tag="u1", name=f"u1_{mt}")
                u2 = pool.tile([128, K // 4], F32, tag="u2", name=f"u2_{mt}")
                thr = pool.tile([128, K // 4], F32, tag="thr", name=f"thr_{mt}")
                nc.vector.tensor_tensor(out=u1[:], in0=pr2[:, :, 0], in1=pr2[:, :, 1], op=ALU.min)
                nc.vector.tensor_tensor(out=u2[:], in0=qs2[:, :, 0], in1=qs2[:, :, 1], op=ALU.max)
                nc.vector.tensor_tensor(out=thr[:], in0=u1[:], in1=u2[:], op=ALU.max)

                rmax = pool.tile([128, 1], F32, tag="sE", bufs=2, name=f"rmax_{mt}")
                nc.vector.tensor_reduce(out=rmax[:], in_=pr[:],
                                        axis=mybir.AxisListType.X, op=ALU.max)

                mask = pool.tile([128, K], FP8, tag="mask", name=f"mask_{mt}")
                r4 = r[:].rearrange("p (g four) -> p g four", four=4)
                m4 = mask[:].rearrange("p (g four) -> p g four", four=4)
                for i in range(4):
                    nc.vector.tensor_tensor(out=m4[:, :, i], in0=r4[:, :, i],
                                            in1=thr[:], op=ALU.is_ge)
                nc.vector.tensor_tensor(out=r[:], in0=r[:], in1=mask[:], op=ALU.mult)

                rm2 = pool.tile([128, 1], F32, tag="sF", bufs=2, name=f"rm2_{mt}")
                nc.vector.tensor_scalar_max(out=rm2[:], in0=rmax[:], scalar1=1e-5)
                rrec = pool.tile([128, 1], F32, tag="sG", bufs=2, name=f"rrec_{mt}")
                nc.vector.reciprocal(out=rrec[:], in_=rm2[:])
                sq = pool.tile([128, 1], F32, tag="sH", bufs=2, name=f"sq_{mt}")
                nc.vector.tensor_scalar_mul(out=sq[:], in0=rrec[:], scalar1=SQRT224)
                xsc = pool.tile([128, 1], F32, tag="sI", bufs=2, name=f"xsc_{mt}")
                nc.vector.tensor_tensor(out=xsc[:], in0=rmax[:], in1=rmax[:], op=ALU.mult)
                xsc2 = pool.tile([128, 1], F32, tag="sJ", bufs=2, name=f"xsc2_{mt}")
                nc.vector.tensor_scalar_mul(out=xsc2[:], in0=xsc[:], scalar1=INV224)
                xsh = pool.tile([128, 1], F32, tag="sK", bufs=2, name=f"xsh_{mt}")
                nc.vector.stream_shuffle(out=xsh[:], in_=xsc2[:], mask=rev32)
                xshs[mt] = xsh

                xq = pool.tile([128, K], FP8, tag="xq8", bufs=2, name=f"xq_{mt}")
                nc.scalar.activation(out=xq[:], in_=r[:], func=ACTF.Square, scale=sq[:])
                xqs[mt] = xq

            def mm_mm(mt):
                xq = xqs[mt]
                xqT = pool.tile([128, KB, 128], BF16, tag="xqT", bufs=3,
                                name=f"xqT_{mt}")
                nc.sync.dma_start_transpose(xqT[:], xq[:].bitcast(BF16))
                xqts[mt] = xqT
                xnat = pool.tile([128, 1], F32, tag="sL", bufs=2, name=f"xnat_{mt}")
                xsh = xshs[mt]
                for q in range(4):
                    nc.sync.dma_start(out=xnat[32 * (3 - q):32 * (4 - q)],
                                      in_=xsh[32 * q:32 * (q + 1)])
                xnats[mt] = xnat
                accs = [psp.tile([128, 512], F32, tag="acc", name=f"acc_{mt}_{ch}")
                        for ch in range(4)]
                wq8 = WqT[:].bitcast(FP8)  # [128, KB, 2*NS]
                xq8 = xqT[:].bitcast(FP8)  # [128, KB, 256]
                for blk in range(KB):
                    lhs = xq8[:, blk, :]
                    for ch in range(4):
                        rhs = wq8[:, blk, ch * 1024:(ch + 1) * 1024].rearrange(
                            "p (n two) -> p two n", two=2)
                        nc.tensor.matmul(accs[ch][:], lhs, rhs,
                                         start=(blk == 0), stop=(blk == KB - 1),
                                         perf_mode=mybir.MatmulPerfMode.DoubleRowSwInterleave)
                return accs

            def dequant(mt, accs):
                xnat = xnats[mt]
                ost = pool.tile([128, NS], BF16, tag="ost", bufs=1, name=f"ost_{mt}")
                for ch in range(4):
                    nc.vector.scalar_tensor_tensor(
                        out=ost[:, ch * 512:(ch + 1) * 512],
                        in0=accs[ch][:], scalar=xnat[:],
                        in1=WscaleB[:, ch * 512:(ch + 1) * 512],
                        op0=ALU.mult, op1=ALU.mult)
                nc.sync.dma_start(out=out[mt * 128:(mt + 1) * 128], in_=ost[:])

            # ---- emission schedule (single DMA ring, software-pipelined) ----
            x_pre(0)
            x_pre(1)
            for wt in range(WT):
                w_chain(wt)
            for ch in range(4):
                psb = psp.tile([128, 512], F32, tag="acc", name=f"psb_{ch}")
                nc.tensor.matmul(psb[:], ones1[:],
                                 wsrow[0:1, ch * 512:(ch + 1) * 512],
                                 start=True, stop=True)
                nc.vector.tensor_copy(out=WscaleB[:, ch * 512:(ch + 1) * 512],
                                      in_=psb[:])

            pending = {}
            for mt in range(NT):
                x_post(mt)
                if mt + 2 < NT:
                    x_pre(mt + 2)
                pending[mt] = mm_mm(mt)
                if mt >= 1:
                    dequant(mt - 1, pending.pop(mt - 1))
            dequant(NT - 1, pending.pop(NT - 1))

    return nc


_cached_nc = None


def _get_nc():
    global _cached_nc
    if _cached_nc is None:
        _cached_nc = _build_program()
    return _cached_nc


def _run(x, W, trace=False):
    x = np.ascontiguousarray(x, dtype=np.float32)
    W = np.ascontiguousarray(W, dtype=np.float32)
    assert x.shape == (M, K) and W.shape == (NW, K)
    nc = _get_nc()
    in_maps = []
    for c in range(N_CORES):
        g, h = c % MG, c // MG
        xsh = x[g * MS:(g + 1) * MS].reshape(NT, 128, K)[:, ::-1, :].reshape(MS, K)
        in_maps.append({
            "xs": np.ascontiguousarray(xsh),
            "ws": W[h * NS:(h + 1) * NS],
        })
    res = run_bass_kernel_spmd(nc, in_maps, core_ids=list(range(N_CORES)),
                               trace=trace)
    outf = np.empty((M, NW), dtype=ml_dtypes.bfloat16)
    for c in range(N_CORES):
        g, h = c % MG, c // MG
        outf[g * MS:(g + 1) * MS, h * NS:(h + 1) * NS] = res.results[c]["out"]
    return outf, res


def kernel(x, W):
    out, _ = _run(x, W, trace=False)
    return out
